# revision 12
# baseline (speedup 1.0000x reference)
"""GCN 2-layer classifier on 8 Trainium2 NeuronCores.

Strategy (node partitioning + replicated bf16 message tables):
  - Nodes are partitioned contiguously across the 8 cores (6250 each).
  - Per layer, each core computes the dense per-node transform for its
    shard ((x@W1)*dinv resp. (relu(agg)+b)@W2*dinv) in bf16, an AllGather
    replicates the compact bf16 "message table" ([rows, 64] bf16, half
    the f32 wire bytes) to every core's HBM, and each core aggregates
    messages for its own destination nodes.
  - Aggregation: edges are sorted by destination block (128 dst nodes)
    and source-row parity, chunked into 128-edge chunks. dma_gather
    requires 256B elements and row pitch, so the compact bf16 table
    (128B rows) is gathered in PAIRS: index j fetches rows {2j, 2j+1}
    (256B); odd-parity chunks use a gather view offset by 64 elements so
    the needed row is always the first 64 gathered columns. Pair indices
    are < 25088 and fit int16 directly.
  - For each chunk a 128x128 scatter matrix sel[e, d] =
    (dst_local[e] == d) * dinv[dst[e]] is built with one DVE
    tensor_scalar against an iota tile, and psum[64, 128] += msgs.T @ sel
    accumulates on the TensorEngine (bf16 operands, f32 PSUM).
  - The symmetric normalization D^-1/2 (A+I) D^-1/2 is factorized:
    source-side dinv is folded into the table rows, dest-side dinv into
    the sel matrix. Self-loops use host-precomputed per-block diagonal
    matrices (onehot * dinv) fed as a constant input.
"""

import sys

if "/opt/trn_rl_repo" not in sys.path:
    sys.path.insert(0, "/opt/trn_rl_repo")

import numpy as np


class _Cfg:
    def __init__(self, n_nodes, in_dim, hid, ncores, group_blocks):
        P = 128
        self.N = n_nodes
        self.IN = in_dim
        self.HID = hid
        self.NCORES = ncores
        self.P = P
        assert n_nodes % ncores == 0
        self.NSH = n_nodes // ncores  # owned nodes per core
        self.NB = -(-self.NSH // P)  # dst blocks per core
        self.NSH_PAD = self.NB * P
        self.TBL = ncores * self.NSH_PAD  # replicated table rows
        self.PAD_STRIDE = self.NSH_PAD - self.NSH  # table row adjustment
        self.GB = group_blocks  # blocks per gather group
        assert self.NB % group_blocks == 0
        self.NG = self.NB // group_blocks
        assert self.TBL // 2 < 32768  # pair indices fit int16


CFG_FULL = _Cfg(n_nodes=50000, in_dim=256, hid=64, ncores=8, group_blocks=7)


def _prep(edge_index, cfg):
    """Host-side index preprocessing: degrees, normalization scalars, and
    per-core chunked edge layouts for the gather/scatter machinery."""
    c = cfg
    src = np.asarray(edge_index[0], dtype=np.int64)
    dst = np.asarray(edge_index[1], dtype=np.int64)
    deg = (np.bincount(dst, minlength=c.N) + 1).astype(np.float32)
    dinv = deg**-0.5  # deg >= 1 everywhere (self-loops)

    # self-loops are NOT materialized as edges: their source rows are local
    # to the owning core and are applied with a per-block diagonal matmul
    s2 = src
    d2 = dst
    trow = s2 + c.PAD_STRIDE * (s2 // c.NSH)  # row in the replicated table
    half = trow & 1  # src-row parity selects the gather view
    core = d2 // c.NSH
    rel = d2 - core * c.NSH
    blk = rel // c.P
    dloc = rel % c.P

    # chunk counts per (block, parity): max over cores so the SPMD program
    # structure is identical on every core
    key = (core * c.NB + blk) * 2 + half
    cnt = np.bincount(key, minlength=c.NCORES * c.NB * 2).reshape(c.NCORES, c.NB, 2)
    cnt_ch = -(-cnt.max(axis=0) // c.P)  # [NB, 2] chunks

    # section layout: group asc -> parity asc -> block asc
    base_ch = np.zeros((c.NB, 2), np.int64)
    acc = 0
    for g in range(c.NG):
        for h in (0, 1):
            for b in range(g * c.GB, (g + 1) * c.GB):
                base_ch[b, h] = acc
                acc += cnt_ch[b, h]
    totc = acc

    per_core = []
    for r in range(c.NCORES):
        m = core == r
        hb, bb, db, tb, d2b = half[m], blk[m], dloc[m], trow[m], d2[m]
        order_key = (bb // c.GB) * (2 * c.NB) + hb * c.NB + bb
        o = np.argsort(order_key, kind="stable")
        hb, bb, db, tb, d2b = hb[o], bb[o], db[o], tb[o], d2b[o]
        ok = order_key[o]
        # rank within each (block, parity) run
        starts = np.r_[0, np.flatnonzero(np.diff(ok)) + 1]
        run_id = np.cumsum(np.r_[0, np.diff(ok) != 0])
        rank = np.arange(len(ok)) - starts[run_id]
        e_pos = base_ch[bb, hb] * c.P + rank
        assert (rank < cnt_ch[bb, hb] * c.P).all()

        idx_flat = np.zeros(totc * c.P, np.int16)
        idx_flat[e_pos] = (tb >> 1).astype(np.int16)  # pair index
        idx16 = idx_flat.reshape(totc * 8, 16).T.copy()  # [16, totc*8]

        dstf_flat = np.full(totc * c.P, 255.0, np.float32)
        dstf_flat[e_pos] = db.astype(np.float32)
        dstf = dstf_flat.reshape(totc, c.P).T.copy()  # [128, totc]

        dvd_flat = np.zeros(totc * c.P, np.float32)
        dvd_flat[e_pos] = dinv[d2b]
        dinvdst = dvd_flat.reshape(totc, c.P).T.copy()

        dv_flat = np.zeros(c.NB * c.P, np.float32)
        dv_flat[: c.NSH] = dinv[r * c.NSH : (r + 1) * c.NSH]
        dv_own = dv_flat.reshape(c.NB, c.P).T.copy()

        # host-precomputed self-loop diag blocks: diag_all[:, b*128+j] is
        # onehot(row==j) * dinv_own[j of block b]
        diag_all = np.zeros((c.P, c.NB * c.P), np.float32)
        for b in range(c.NB):
            np.fill_diagonal(diag_all[:, b * c.P : (b + 1) * c.P], dv_own[:, b])

        per_core.append(
            {
                "idx16": idx16,
                "dstf": dstf,
                "dinvdst": dinvdst,
                "dinv_own": dv_own,
                "diag_all": diag_all,
            }
        )

    return per_core, cnt_ch, base_ch, totc


def _build(cfg, cnt_ch, base_ch, totc, bc_val, repeat=1):
    import concourse.bacc as bacc
    import concourse.mybir as mybir
    import concourse.tile as tile

    c = cfg
    P, HID, NB = c.P, c.HID, c.NB
    f32 = mybir.dt.float32
    bf16 = mybir.dt.bfloat16

    nc = bacc.Bacc(
        "TRN2",
        target_bir_lowering=False,
        debug=False,
        num_devices=c.NCORES,
        num_swdge_queues=4,
    )

    xT_in = nc.dram_tensor("xT", [c.IN, c.NSH_PAD], bf16, kind="ExternalInput")
    idx_in = nc.dram_tensor(
        "idx16", [16, totc * 8], mybir.dt.int16, kind="ExternalInput"
    )
    dstf_in = nc.dram_tensor("dstf", [P, totc], f32, kind="ExternalInput")
    dvd_in = nc.dram_tensor("dinvdst", [P, totc], f32, kind="ExternalInput")
    dvo_in = nc.dram_tensor("dinv_own", [P, NB], f32, kind="ExternalInput")
    iota_in = nc.dram_tensor("iota", [P, P], bf16, kind="ExternalInput")
    diag_in = nc.dram_tensor("diag_all", [P, NB * P], bf16, kind="ExternalInput")
    w1_in = nc.dram_tensor("W1", [c.IN, HID], bf16, kind="ExternalInput")
    w2_in = nc.dram_tensor("W2", [HID, HID], bf16, kind="ExternalInput")
    wc_in = nc.dram_tensor("Wc", [HID, 1], bf16, kind="ExternalInput")
    b1_in = nc.dram_tensor("b1c", [HID, 1], f32, kind="ExternalInput")
    b2_in = nc.dram_tensor("b2c", [HID, 1], f32, kind="ExternalInput")
    out_t = nc.dram_tensor("out", [P, NB], f32, kind="ExternalOutput")

    # flat 1D tables so the odd-parity gather view can sit at a 64-elem offset
    tbl_loc = [
        nc.dram_tensor(f"tbl{i}_loc", [c.NSH_PAD * HID], bf16, kind="Internal")
        for i in (1, 2)
    ]
    tbl_sh = [
        nc.dram_tensor(
            f"tbl{i}_sh", [c.TBL * HID], bf16, kind="Internal", addr_space="Shared"
        )
        for i in (1, 2)
    ]

    # per-(group, parity) gather call geometry
    gbase = np.zeros((c.NG, 2), np.int64)
    gcnt = np.zeros((c.NG, 2), np.int64)
    for g in range(c.NG):
        for h in (0, 1):
            gbase[g, h] = base_ch[g * c.GB, h]
            gcnt[g, h] = cnt_ch[g * c.GB : (g + 1) * c.GB, h].sum()

    NPAIR = c.TBL // 2

    with tile.TileContext(nc) as tc:
        with (
            tc.tile_pool(name="const", bufs=1) as cp,
            tc.tile_pool(name="work", bufs=12) as wp,
            tc.tile_pool(name="pagg", bufs=4, space="PSUM") as pa,
            tc.tile_pool(name="pdense", bufs=2, space="PSUM") as pd,
            tc.tile_pool(name="plogit", bufs=2, space="PSUM") as pl,
        ):
            # ---- constants ----
            idx16 = cp.tile([P, totc * 8], mybir.dt.int16)
            for rep in range(8):
                nc.sync.dma_start(
                    out=idx16[16 * rep : 16 * (rep + 1), :], in_=idx_in[:, :]
                )
            dstf = cp.tile([P, totc], f32)
            nc.sync.dma_start(out=dstf[:], in_=dstf_in[:, :])
            dinvdst = cp.tile([P, totc], f32)
            nc.sync.dma_start(out=dinvdst[:], in_=dvd_in[:, :])
            dinv_own = cp.tile([P, NB], f32)
            nc.sync.dma_start(out=dinv_own[:], in_=dvo_in[:, :])
            iota = cp.tile([P, P], bf16)
            nc.sync.dma_start(out=iota[:], in_=iota_in[:, :])
            diag_all = cp.tile([P, NB * P], bf16)
            nc.sync.dma_start(out=diag_all[:], in_=diag_in[:, :])
            w1a = cp.tile([P, HID], bf16)
            nc.sync.dma_start(out=w1a[:], in_=w1_in[0:P, :])
            w1b = cp.tile([P, HID], bf16)
            nc.sync.dma_start(out=w1b[:], in_=w1_in[P : 2 * P, :])
            w2 = cp.tile([HID, HID], bf16)
            nc.sync.dma_start(out=w2[:], in_=w2_in[:, :])
            wc = cp.tile([HID, 1], bf16)
            nc.sync.dma_start(out=wc[:], in_=wc_in[:, :])
            b1c = cp.tile([HID, 1], f32)
            nc.sync.dma_start(out=b1c[:], in_=b1_in[:, :])
            b2c = cp.tile([HID, 1], f32)
            nc.sync.dma_start(out=b2c[:], in_=b2_in[:, :])

            # persistent per-layer node results [128, NB*HID] bf16: col block
            # b holds (table row for node b*128+p); feeds the self-loop matmul
            # without re-reading HBM and batches the table writes per group
            sck = [cp.tile([P, NB * HID], bf16, name=f"sck{i}") for i in (0, 1)]
            res_all = cp.tile([P, NB], f32)

            def store_group(layer, g):
                """One strided DMA for a group's NB-block stripe of the
                local table shard (instead of GB small row writes)."""
                r0, r1 = g * c.GB * P, (g + 1) * c.GB * P
                c0, c1 = g * c.GB * HID, (g + 1) * c.GB * HID
                nc.sync.dma_start(
                    out=tbl_loc[layer][r0 * HID : r1 * HID].rearrange(
                        "(b p e) -> p b e", p=P, e=HID
                    ),
                    in_=sck[layer][:, c0:c1].rearrange("p (b e) -> p b e", e=HID),
                )

            for _rep in range(repeat):
                # ---- phase B: table1 = (x @ W1) * dinv for owned nodes ----
                with tc.tile_pool(name="xt", bufs=1) as xp:
                    # per-group column loads: subtile deps let block b's
                    # matmuls start after its group's stripe lands instead
                    # of waiting for the whole tile
                    xt0 = xp.tile([P, c.NSH_PAD], bf16)
                    xt1 = xp.tile([P, c.NSH_PAD], bf16)
                    for g in range(c.NG):
                        gs = slice(g * c.GB * P, (g + 1) * c.GB * P)
                        nc.sync.dma_start(out=xt0[:, gs], in_=xT_in[0:P, gs])
                        nc.sync.dma_start(out=xt1[:, gs], in_=xT_in[P : 2 * P, gs])
                    for b in range(NB):
                        bs = slice(b * P, (b + 1) * P)
                        ph = pd.tile([P, HID], f32, tag="pdense")
                        nc.tensor.matmul(
                            out=ph[:], lhsT=xt0[:, bs], rhs=w1a[:], start=True, stop=False
                        )
                        nc.tensor.matmul(
                            out=ph[:], lhsT=xt1[:, bs], rhs=w1b[:], start=False, stop=True
                        )
                        nc.vector.tensor_scalar(
                            out=sck[0][:, b * HID : (b + 1) * HID],
                            in0=ph[:],
                            scalar1=dinv_own[:, b : b + 1],
                            scalar2=None,
                            op0=mybir.AluOpType.mult,
                        )
                        if (b + 1) % c.GB == 0:
                            store_group(0, b // c.GB)

                qrr = [0]  # rolling queue assignment across all gather sub-calls

                def aggregate(mps, layer, emit_block, emit_group=None):
                    """Per dst block: psum[64, 128] = sum over edges of
                    table[src] (x) sel, then emit_block(b, psum).

                    A single pair view serves both parities: odd rows have
                    trow>>1 == (trow-1)>>1, so the needed row is the second
                    64 columns of the gathered pair — chunk parity selects
                    the lhsT column offset. Gather sub-calls therefore span
                    the group's full (parity 0 + parity 1) chunk range."""
                    tbl = tbl_sh[layer]
                    view = tbl[0 : NPAIR * 2 * HID].rearrange("(p e) -> p e", e=2 * HID)
                    for g in range(c.NG):
                        n = int(gcnt[g, 0] + gcnt[g, 1])
                        t = mps.tile([P, n * 2 * HID], bf16, tag="ms")
                        i0 = int(gbase[g, 0]) * 8
                        # sub-calls of <=1024 idxs with single_packet=True
                        # (one max-size packet per SDMA engine: ~3x faster
                        # than per-descriptor packets; >1024 idxs would
                        # crash), round-robin over the 4 SWDGE queues
                        SUBCH = 8
                        for si in range(-(-n // SUBCH)):
                            s0, s1 = si * SUBCH, min((si + 1) * SUBCH, n)
                            nc.gpsimd.dma_gather(
                                out_ap=t[:, s0 * 2 * HID : s1 * 2 * HID].rearrange(
                                    "p (c e) -> p c e", e=2 * HID
                                ),
                                in_ap=view,
                                idxs_ap=idx16[:, i0 + s0 * 8 : i0 + s1 * 8],
                                num_idxs=(s1 - s0) * P,
                                num_idxs_reg=(s1 - s0) * P,
                                elem_size=2 * HID,
                                single_packet=True,
                                queue_num=qrr[0] % 4,
                            )
                            qrr[0] += 1
                        for b in range(g * c.GB, (g + 1) * c.GB):
                            chunks = []
                            for h in (0, 1):
                                for k in range(int(cnt_ch[b, h])):
                                    cg = int(base_ch[b, h]) + k
                                    cl = cg - int(gbase[g, 0])
                                    chunks.append((cg, t, cl, h))
                            pt = pa.tile([HID, P], f32, tag="pagg")
                            # self-loop contribution: psum += own_rows.T @ diag(dinv)
                            nc.tensor.matmul(
                                out=pt[:],
                                lhsT=sck[layer][:, b * HID : (b + 1) * HID],
                                rhs=diag_all[:, b * P : (b + 1) * P],
                                start=True,
                                stop=(len(chunks) == 0),
                            )
                            for j, (cg, mt, cl, h) in enumerate(chunks):
                                sel = wp.tile([P, P], bf16, tag="sel")
                                nc.vector.tensor_scalar(
                                    out=sel[:],
                                    in0=iota[:],
                                    scalar1=dstf[:, cg : cg + 1],
                                    scalar2=dinvdst[:, cg : cg + 1],
                                    op0=mybir.AluOpType.is_equal,
                                    op1=mybir.AluOpType.mult,
                                )
                                o0 = cl * 2 * HID + h * HID
                                nc.tensor.matmul(
                                    out=pt[:],
                                    lhsT=mt[:, o0 : o0 + HID],
                                    rhs=sel[:],
                                    start=False,
                                    stop=(j == len(chunks) - 1),
                                )
                            emit_block(b, pt)
                        if emit_group is not None:
                            emit_group(g)

                # ---- allgather table1, aggregate, table2 = relu(agg)+b1 @ W2 ----
                nc.gpsimd.collective_compute(
                    "AllGather",
                    mybir.AluOpType.bypass,
                    replica_groups=[list(range(c.NCORES))],
                    ins=[tbl_loc[0][:].opt()],
                    outs=[tbl_sh[0][:].opt()],
                )

                def emit_layer1(b, pt):
                    o1t = wp.tile([HID, P], bf16, tag="o1t")
                    nc.scalar.activation(
                        out=o1t[:],
                        in_=pt[:],
                        func=mybir.ActivationFunctionType.Relu,
                        bias=b1c[:, 0:1],
                    )
                    ph2 = pd.tile([P, HID], f32, tag="pdense")
                    nc.tensor.matmul(
                        out=ph2[:], lhsT=o1t[:], rhs=w2[:], start=True, stop=True
                    )
                    nc.vector.tensor_scalar(
                        out=sck[1][:, b * HID : (b + 1) * HID],
                        in0=ph2[:],
                        scalar1=dinv_own[:, b : b + 1],
                        scalar2=None,
                        op0=mybir.AluOpType.mult,
                    )

                def emit_layer2(b, pt):
                    o2t = wp.tile([HID, P], bf16, tag="o1t")
                    nc.scalar.activation(
                        out=o2t[:],
                        in_=pt[:],
                        func=mybir.ActivationFunctionType.Relu,
                        bias=b2c[:, 0:1],
                    )
                    plg = pl.tile([P, 1], f32, tag="plogit")
                    nc.tensor.matmul(
                        out=plg[:], lhsT=o2t[:], rhs=wc[:], start=True, stop=True
                    )
                    nc.scalar.activation(
                        out=res_all[:, b : b + 1],
                        in_=plg[:],
                        func=mybir.ActivationFunctionType.Sigmoid,
                        bias=float(bc_val),
                    )

                with tc.tile_pool(name="msgs", bufs=4) as mp:
                    aggregate(
                        mp,
                        0,
                        emit_layer1,
                        emit_group=lambda g: store_group(1, g),
                    )

                    # ---- allgather table2, aggregate, classifier ----
                    nc.gpsimd.collective_compute(
                        "AllGather",
                        mybir.AluOpType.bypass,
                        replica_groups=[list(range(c.NCORES))],
                        ins=[tbl_loc[1][:].opt()],
                        outs=[tbl_sh[1][:].opt()],
                    )

                    aggregate(mp, 1, emit_layer2)

                nc.sync.dma_start(out=out_t[:, :], in_=res_all[:])

    nc.compile()
    return nc


def _to_bf16(a):
    import ml_dtypes

    return np.asarray(a, dtype=np.float32).astype(ml_dtypes.bfloat16)


def _make_in_maps(x, W1, W2, Wc, b1, b2, per_core, cfg):
    c = cfg
    iota = np.tile(np.arange(c.P, dtype=np.float32), (c.P, 1))
    in_maps = []
    for r in range(c.NCORES):
        xs = np.asarray(x, dtype=np.float32)[r * c.NSH : (r + 1) * c.NSH]
        xT = np.zeros((c.IN, c.NSH_PAD), np.float32)
        xT[:, : c.NSH] = xs.T
        pc = per_core[r]
        in_maps.append(
            {
                "xT": _to_bf16(xT),
                "idx16": pc["idx16"],
                "dstf": pc["dstf"],
                "dinvdst": pc["dinvdst"],
                "dinv_own": pc["dinv_own"],
                "iota": _to_bf16(iota),
                "diag_all": _to_bf16(pc["diag_all"]),
                "W1": _to_bf16(W1),
                "W2": _to_bf16(W2),
                "Wc": _to_bf16(Wc),
                "b1c": np.asarray(b1, np.float32).reshape(c.HID, 1),
                "b2c": np.asarray(b2, np.float32).reshape(c.HID, 1),
            }
        )
    return in_maps


def _assemble(results, cfg):
    c = cfg
    # out is [128, NB]: node b*128+p at [p, b]
    return np.concatenate(
        [results[r]["out"].T.reshape(-1, 1)[: c.NSH] for r in range(c.NCORES)],
        axis=0,
    )


def _make_runner(nc, in_maps, n_cores):
    """Build a reusable sharded PJRT callable over device-resident inputs.
    Returns (run_once() -> list[dict], time_steady(iters) -> ns_per_iter)."""
    import time as _t

    import jax
    import numpy as np_
    from jax.sharding import Mesh, NamedSharding, PartitionSpec
    from jax.experimental.shard_map import shard_map

    import concourse.mybir as mybir
    from concourse.bass2jax import _bass_exec_p, install_neuronx_cc_hook

    install_neuronx_cc_hook()

    partition_name = nc.partition_id_tensor.name if nc.partition_id_tensor else None
    in_names, out_names, out_avals, zero_outs = [], [], [], []
    for alloc in nc.m.functions[0].allocations:
        if not isinstance(alloc, mybir.MemoryLocationSet):
            continue
        name = alloc.memorylocations[0].name
        if alloc.kind == "ExternalInput":
            if name != partition_name:
                in_names.append(name)
        elif alloc.kind == "ExternalOutput":
            out_names.append(name)
            shape = tuple(alloc.tensor_shape)
            dtype = mybir.dt.np(alloc.dtype)
            out_avals.append(jax.core.ShapedArray(shape, dtype))
            zero_outs.append(np_.zeros(shape, dtype))
    n_params = len(in_names)
    all_in_names = in_names + out_names
    if partition_name is not None:
        all_in_names = all_in_names + [partition_name]

    def _body(*args):
        operands = list(args)
        if partition_name is not None:
            from concourse.bass2jax import partition_id_tensor

            operands.append(partition_id_tensor())
        outs = _bass_exec_p.bind(
            *operands,
            out_avals=tuple(out_avals),
            in_names=tuple(all_in_names),
            out_names=tuple(out_names),
            lowering_input_output_aliases=(),
            sim_require_finite=True,
            sim_require_nnan=True,
            nc=nc,
        )
        return tuple(outs)

    devices = jax.devices()[:n_cores]
    mesh = Mesh(np_.asarray(devices), ("core",))
    in_specs = (PartitionSpec("core"),) * (n_params + len(out_names))
    out_specs = (PartitionSpec("core"),) * len(out_names)
    sharded = jax.jit(
        shard_map(
            _body, mesh=mesh, in_specs=in_specs, out_specs=out_specs, check_rep=False
        ),
        keep_unused=True,
    )
    sh = NamedSharding(mesh, PartitionSpec("core"))
    concat_in = [
        jax.device_put(
            np_.concatenate([np_.asarray(in_maps[c][nm]) for c in range(n_cores)], 0),
            sh,
        )
        for nm in in_names
    ]
    concat_zeros = [
        jax.device_put(np_.zeros((n_cores * z.shape[0], *z.shape[1:]), z.dtype), sh)
        for z in zero_outs
    ]

    def run_once():
        out_arrs = sharded(*concat_in, *concat_zeros)
        jax.block_until_ready(out_arrs)
        return [
            {
                nm: np_.asarray(out_arrs[i]).reshape(n_cores, *out_avals[i].shape)[cc]
                for i, nm in enumerate(out_names)
            }
            for cc in range(n_cores)
        ]

    def time_steady(iters=10, warmup=3):
        for _ in range(warmup):
            jax.block_until_ready(sharded(*concat_in, *concat_zeros))
        t0 = _t.perf_counter()
        last = None
        for _ in range(iters):
            last = sharded(*concat_in, *concat_zeros)
        jax.block_until_ready(last)
        t1 = _t.perf_counter()
        return (t1 - t0) / iters * 1e9

    return run_once, time_steady


_CACHE = {}


def _get_built(edge_index, bc, repeat):
    """Cache (prep, compiled nc) keyed by edge data + bc + repeat."""
    edges = np.asarray(edge_index, dtype=np.int64)
    key = (hash(edges.tobytes()), float(bc), int(repeat))
    if key not in _CACHE:
        per_core, cnt_ch, base_ch, totc = _prep(edges, CFG_FULL)
        nc = _build(CFG_FULL, cnt_ch, base_ch, totc, float(bc), repeat=repeat)
        _CACHE[key] = (per_core, nc)
    return _CACHE[key]


def kernel(x, edge_index, W1, b1, W2, b2, Wc, bc):
    from concourse import bass_utils

    c = CFG_FULL
    bc_val = float(np.asarray(bc).reshape(-1)[0])
    per_core, nc = _get_built(edge_index, bc_val, 1)
    in_maps = _make_in_maps(x, W1, W2, Wc, b1, b2, per_core, c)
    res = bass_utils.run_bass_kernel_spmd(
        nc, in_maps, core_ids=list(range(c.NCORES)), trace=False
    )
    return _assemble(res.results, c)


# revision 13
# speedup vs baseline: 1.1601x; 1.1601x over previous
"""GCN 2-layer classifier on 8 Trainium2 NeuronCores.

Strategy (node partitioning + replicated bf16 message tables):
  - Nodes are partitioned contiguously across the 8 cores (6250 each).
  - Per layer, each core computes the dense per-node transform for its
    shard ((x@W1)*dinv resp. (relu(agg)+b)@W2*dinv) in bf16, an AllGather
    replicates the compact bf16 "message table" ([rows, 64] bf16, half
    the f32 wire bytes) to every core's HBM, and each core aggregates
    messages for its own destination nodes.
  - Aggregation: edges are sorted by destination block (128 dst nodes)
    and source-row parity, chunked into 128-edge chunks. dma_gather
    requires 256B elements and row pitch, so the compact bf16 table
    (128B rows) is gathered in PAIRS: index j fetches rows {2j, 2j+1}
    (256B); odd-parity chunks use a gather view offset by 64 elements so
    the needed row is always the first 64 gathered columns. Pair indices
    are < 25088 and fit int16 directly.
  - For each chunk a 128x128 scatter matrix sel[e, d] =
    (dst_local[e] == d) * dinv[dst[e]] is built with one DVE
    tensor_scalar against an iota tile, and psum[64, 128] += msgs.T @ sel
    accumulates on the TensorEngine (bf16 operands, f32 PSUM).
  - The symmetric normalization D^-1/2 (A+I) D^-1/2 is factorized:
    source-side dinv is folded into the table rows, dest-side dinv into
    the sel matrix. Self-loops use host-precomputed per-block diagonal
    matrices (onehot * dinv) fed as a constant input.
"""

import sys

if "/opt/trn_rl_repo" not in sys.path:
    sys.path.insert(0, "/opt/trn_rl_repo")

import numpy as np


class _Cfg:
    def __init__(self, n_nodes, in_dim, hid, ncores, group_blocks):
        P = 128
        self.N = n_nodes
        self.IN = in_dim
        self.HID = hid
        self.NCORES = ncores
        self.P = P
        assert n_nodes % ncores == 0
        self.NSH = n_nodes // ncores  # owned nodes per core
        self.NB = -(-self.NSH // P)  # dst blocks per core
        self.NSH_PAD = self.NB * P
        self.TBL = ncores * self.NSH_PAD  # replicated table rows
        self.PAD_STRIDE = self.NSH_PAD - self.NSH  # table row adjustment
        self.GB = group_blocks  # blocks per gather group
        assert self.NB % group_blocks == 0
        self.NG = self.NB // group_blocks
        assert self.TBL // 2 < 32768  # pair indices fit int16


CFG_FULL = _Cfg(n_nodes=50000, in_dim=256, hid=64, ncores=8, group_blocks=7)


def _prep(edge_index, cfg):
    """Host-side index preprocessing: degrees, normalization scalars, and
    per-core chunked edge layouts for the gather/scatter machinery."""
    c = cfg
    src = np.asarray(edge_index[0], dtype=np.int64)
    dst = np.asarray(edge_index[1], dtype=np.int64)
    deg = (np.bincount(dst, minlength=c.N) + 1).astype(np.float32)
    dinv = deg**-0.5  # deg >= 1 everywhere (self-loops)

    # self-loops are NOT materialized as edges: their source rows are local
    # to the owning core and are applied with a per-block diagonal matmul
    s2 = src
    d2 = dst
    trow = s2 + c.PAD_STRIDE * (s2 // c.NSH)  # row in the replicated table
    half = trow & 1  # src-row parity selects the gather view
    core = d2 // c.NSH
    rel = d2 - core * c.NSH
    blk = rel // c.P
    dloc = rel % c.P

    # chunk counts per (block, parity): max over cores so the SPMD program
    # structure is identical on every core
    key = (core * c.NB + blk) * 2 + half
    cnt = np.bincount(key, minlength=c.NCORES * c.NB * 2).reshape(c.NCORES, c.NB, 2)
    cnt_ch = -(-cnt.max(axis=0) // c.P)  # [NB, 2] chunks

    # section layout: group asc -> parity asc -> block asc
    base_ch = np.zeros((c.NB, 2), np.int64)
    acc = 0
    for g in range(c.NG):
        for h in (0, 1):
            for b in range(g * c.GB, (g + 1) * c.GB):
                base_ch[b, h] = acc
                acc += cnt_ch[b, h]
    totc = acc

    per_core = []
    for r in range(c.NCORES):
        m = core == r
        hb, bb, db, tb, d2b = half[m], blk[m], dloc[m], trow[m], d2[m]
        order_key = (bb // c.GB) * (2 * c.NB) + hb * c.NB + bb
        o = np.argsort(order_key, kind="stable")
        hb, bb, db, tb, d2b = hb[o], bb[o], db[o], tb[o], d2b[o]
        ok = order_key[o]
        # rank within each (block, parity) run
        starts = np.r_[0, np.flatnonzero(np.diff(ok)) + 1]
        run_id = np.cumsum(np.r_[0, np.diff(ok) != 0])
        rank = np.arange(len(ok)) - starts[run_id]
        e_pos = base_ch[bb, hb] * c.P + rank
        assert (rank < cnt_ch[bb, hb] * c.P).all()

        idx_flat = np.zeros(totc * c.P, np.int16)
        idx_flat[e_pos] = (tb >> 1).astype(np.int16)  # pair index
        idx16 = idx_flat.reshape(totc * 8, 16).T.copy()  # [16, totc*8]

        dstf_flat = np.full(totc * c.P, 255.0, np.float32)
        dstf_flat[e_pos] = db.astype(np.float32)
        dstf = dstf_flat.reshape(totc, c.P).T.copy()  # [128, totc]

        dvd_flat = np.zeros(totc * c.P, np.float32)
        dvd_flat[e_pos] = dinv[d2b]
        dinvdst = dvd_flat.reshape(totc, c.P).T.copy()

        dv_flat = np.zeros(c.NB * c.P, np.float32)
        dv_flat[: c.NSH] = dinv[r * c.NSH : (r + 1) * c.NSH]
        dv_own = dv_flat.reshape(c.NB, c.P).T.copy()

        # host-precomputed self-loop diag blocks: diag_all[:, b*128+j] is
        # onehot(row==j) * dinv_own[j of block b]
        diag_all = np.zeros((c.P, c.NB * c.P), np.float32)
        for b in range(c.NB):
            np.fill_diagonal(diag_all[:, b * c.P : (b + 1) * c.P], dv_own[:, b])

        per_core.append(
            {
                "idx16": idx16,
                "dstf": dstf,
                "dinvdst": dinvdst,
                "dinv_own": dv_own,
                "diag_all": diag_all,
            }
        )

    return per_core, cnt_ch, base_ch, totc


def _build(cfg, cnt_ch, base_ch, totc, bc_val, repeat=1):
    import concourse.bacc as bacc
    import concourse.mybir as mybir
    import concourse.tile as tile

    c = cfg
    P, HID, NB = c.P, c.HID, c.NB
    f32 = mybir.dt.float32
    bf16 = mybir.dt.bfloat16

    nc = bacc.Bacc(
        "TRN2",
        target_bir_lowering=False,
        debug=False,
        num_devices=c.NCORES,
        num_swdge_queues=4,
    )

    xT_in = nc.dram_tensor("xT", [c.IN, c.NSH_PAD], bf16, kind="ExternalInput")
    idx_in = nc.dram_tensor(
        "idx16", [16, totc * 8], mybir.dt.int16, kind="ExternalInput"
    )
    dstf_in = nc.dram_tensor("dstf", [P, totc], f32, kind="ExternalInput")
    dvd_in = nc.dram_tensor("dinvdst", [P, totc], f32, kind="ExternalInput")
    dvo_in = nc.dram_tensor("dinv_own", [P, NB], f32, kind="ExternalInput")
    iota_in = nc.dram_tensor("iota", [P, P], bf16, kind="ExternalInput")
    diag_in = nc.dram_tensor("diag_all", [P, NB * P], bf16, kind="ExternalInput")
    w1_in = nc.dram_tensor("W1", [c.IN, HID], bf16, kind="ExternalInput")
    w2_in = nc.dram_tensor("W2", [HID, HID], bf16, kind="ExternalInput")
    wc_in = nc.dram_tensor("Wc", [HID, 1], bf16, kind="ExternalInput")
    b1_in = nc.dram_tensor("b1c", [HID, 1], f32, kind="ExternalInput")
    b2_in = nc.dram_tensor("b2c", [HID, 1], f32, kind="ExternalInput")
    out_t = nc.dram_tensor("out", [P, NB], f32, kind="ExternalOutput")

    # flat 1D tables so the odd-parity gather view can sit at a 64-elem offset
    tbl_loc = [
        nc.dram_tensor(f"tbl{i}_loc", [c.NSH_PAD * HID], bf16, kind="Internal")
        for i in (1, 2)
    ]
    tbl_sh = [
        nc.dram_tensor(
            f"tbl{i}_sh", [c.TBL * HID], bf16, kind="Internal", addr_space="Shared"
        )
        for i in (1, 2)
    ]

    # per-(group, parity) gather call geometry
    gbase = np.zeros((c.NG, 2), np.int64)
    gcnt = np.zeros((c.NG, 2), np.int64)
    for g in range(c.NG):
        for h in (0, 1):
            gbase[g, h] = base_ch[g * c.GB, h]
            gcnt[g, h] = cnt_ch[g * c.GB : (g + 1) * c.GB, h].sum()

    NPAIR = c.TBL // 2

    with tile.TileContext(nc) as tc:
        with (
            tc.tile_pool(name="const", bufs=1) as cp,
            tc.tile_pool(name="work", bufs=12) as wp,
            tc.tile_pool(name="pagg", bufs=4, space="PSUM") as pa,
            tc.tile_pool(name="pdense", bufs=2, space="PSUM") as pd,
            tc.tile_pool(name="plogit", bufs=2, space="PSUM") as pl,
        ):
            # ---- constants ----
            idx16 = cp.tile([P, totc * 8], mybir.dt.int16)
            for rep in range(8):
                nc.sync.dma_start(
                    out=idx16[16 * rep : 16 * (rep + 1), :], in_=idx_in[:, :]
                )
            dstf = cp.tile([P, totc], f32)
            nc.sync.dma_start(out=dstf[:], in_=dstf_in[:, :])
            dinvdst = cp.tile([P, totc], f32)
            nc.sync.dma_start(out=dinvdst[:], in_=dvd_in[:, :])
            dinv_own = cp.tile([P, NB], f32)
            nc.sync.dma_start(out=dinv_own[:], in_=dvo_in[:, :])
            iota = cp.tile([P, P], bf16)
            nc.sync.dma_start(out=iota[:], in_=iota_in[:, :])
            diag_all = cp.tile([P, NB * P], bf16)
            nc.sync.dma_start(out=diag_all[:], in_=diag_in[:, :])
            w1a = cp.tile([P, HID], bf16)
            nc.sync.dma_start(out=w1a[:], in_=w1_in[0:P, :])
            w1b = cp.tile([P, HID], bf16)
            nc.sync.dma_start(out=w1b[:], in_=w1_in[P : 2 * P, :])
            w2 = cp.tile([HID, HID], bf16)
            nc.sync.dma_start(out=w2[:], in_=w2_in[:, :])
            wc = cp.tile([HID, 1], bf16)
            nc.sync.dma_start(out=wc[:], in_=wc_in[:, :])
            b1c = cp.tile([HID, 1], f32)
            nc.sync.dma_start(out=b1c[:], in_=b1_in[:, :])
            b2c = cp.tile([HID, 1], f32)
            nc.sync.dma_start(out=b2c[:], in_=b2_in[:, :])

            # persistent per-layer node results [128, NB*HID] bf16: col block
            # b holds (table row for node b*128+p); feeds the self-loop matmul
            # without re-reading HBM and batches the table writes per group
            sck = [cp.tile([P, NB * HID], bf16, name=f"sck{i}") for i in (0, 1)]
            res_all = cp.tile([P, NB], f32)

            def store_group(layer, g):
                """One strided DMA for a group's NB-block stripe of the
                local table shard (instead of GB small row writes)."""
                r0, r1 = g * c.GB * P, (g + 1) * c.GB * P
                c0, c1 = g * c.GB * HID, (g + 1) * c.GB * HID
                nc.sync.dma_start(
                    out=tbl_loc[layer][r0 * HID : r1 * HID].rearrange(
                        "(b p e) -> p b e", p=P, e=HID
                    ),
                    in_=sck[layer][:, c0:c1].rearrange("p (b e) -> p b e", e=HID),
                )

            # persistent x tiles: per-repeat reloads overlap the previous
            # iteration's tail instead of waiting on a fresh pool open
            xt0 = cp.tile([P, c.NSH_PAD], bf16, name="xt0")
            xt1 = cp.tile([P, c.NSH_PAD], bf16, name="xt1")

            for _rep in range(repeat):
                # ---- phase B: table1 = (x @ W1) * dinv for owned nodes ----
                if True:
                    # per-group column loads: subtile deps let block b's
                    # matmuls start after its group's stripe lands instead
                    # of waiting for the whole tile
                    for g in range(c.NG):
                        gs = slice(g * c.GB * P, (g + 1) * c.GB * P)
                        nc.sync.dma_start(out=xt0[:, gs], in_=xT_in[0:P, gs])
                        nc.sync.dma_start(out=xt1[:, gs], in_=xT_in[P : 2 * P, gs])
                    for b in range(NB):
                        bs = slice(b * P, (b + 1) * P)
                        ph = pd.tile([P, HID], f32, tag="pdense")
                        nc.tensor.matmul(
                            out=ph[:], lhsT=xt0[:, bs], rhs=w1a[:], start=True, stop=False
                        )
                        nc.tensor.matmul(
                            out=ph[:], lhsT=xt1[:, bs], rhs=w1b[:], start=False, stop=True
                        )
                        nc.vector.tensor_scalar(
                            out=sck[0][:, b * HID : (b + 1) * HID],
                            in0=ph[:],
                            scalar1=dinv_own[:, b : b + 1],
                            scalar2=None,
                            op0=mybir.AluOpType.mult,
                        )
                        if (b + 1) % c.GB == 0:
                            store_group(0, b // c.GB)

                qrr = [0]  # rolling queue assignment across all gather sub-calls

                def aggregate(mps, layer, emit_block, emit_group=None):
                    """Per dst block: psum[64, 128] = sum over edges of
                    table[src] (x) sel, then emit_block(b, psum).

                    A single pair view serves both parities: odd rows have
                    trow>>1 == (trow-1)>>1, so the needed row is the second
                    64 columns of the gathered pair — chunk parity selects
                    the lhsT column offset. Gather sub-calls therefore span
                    the group's full (parity 0 + parity 1) chunk range."""
                    tbl = tbl_sh[layer]
                    view = tbl[0 : NPAIR * 2 * HID].rearrange("(p e) -> p e", e=2 * HID)
                    for g in range(c.NG):
                        n = int(gcnt[g, 0] + gcnt[g, 1])
                        t = mps.tile([P, n * 2 * HID], bf16, tag="ms")
                        i0 = int(gbase[g, 0]) * 8
                        # sub-calls of <=1024 idxs with single_packet=True
                        # (one max-size packet per SDMA engine: ~3x faster
                        # than per-descriptor packets; >1024 idxs would
                        # crash), round-robin over the 4 SWDGE queues
                        SUBCH = 8
                        for si in range(-(-n // SUBCH)):
                            s0, s1 = si * SUBCH, min((si + 1) * SUBCH, n)
                            nc.gpsimd.dma_gather(
                                out_ap=t[:, s0 * 2 * HID : s1 * 2 * HID].rearrange(
                                    "p (c e) -> p c e", e=2 * HID
                                ),
                                in_ap=view,
                                idxs_ap=idx16[:, i0 + s0 * 8 : i0 + s1 * 8],
                                num_idxs=(s1 - s0) * P,
                                num_idxs_reg=(s1 - s0) * P,
                                elem_size=2 * HID,
                                single_packet=True,
                                queue_num=qrr[0] % 4,
                            )
                            qrr[0] += 1
                        for b in range(g * c.GB, (g + 1) * c.GB):
                            chunks = []
                            for h in (0, 1):
                                for k in range(int(cnt_ch[b, h])):
                                    cg = int(base_ch[b, h]) + k
                                    cl = cg - int(gbase[g, 0])
                                    chunks.append((cg, t, cl, h))
                            pt = pa.tile([HID, P], f32, tag="pagg")
                            # self-loop contribution: psum += own_rows.T @ diag(dinv)
                            nc.tensor.matmul(
                                out=pt[:],
                                lhsT=sck[layer][:, b * HID : (b + 1) * HID],
                                rhs=diag_all[:, b * P : (b + 1) * P],
                                start=True,
                                stop=(len(chunks) == 0),
                            )
                            for j, (cg, mt, cl, h) in enumerate(chunks):
                                sel = wp.tile([P, P], bf16, tag="sel")
                                nc.vector.tensor_scalar(
                                    out=sel[:],
                                    in0=iota[:],
                                    scalar1=dstf[:, cg : cg + 1],
                                    scalar2=dinvdst[:, cg : cg + 1],
                                    op0=mybir.AluOpType.is_equal,
                                    op1=mybir.AluOpType.mult,
                                )
                                o0 = cl * 2 * HID + h * HID
                                nc.tensor.matmul(
                                    out=pt[:],
                                    lhsT=mt[:, o0 : o0 + HID],
                                    rhs=sel[:],
                                    start=False,
                                    stop=(j == len(chunks) - 1),
                                )
                            emit_block(b, pt)
                        if emit_group is not None:
                            emit_group(g)

                # ---- allgather table1, aggregate, table2 = relu(agg)+b1 @ W2 ----
                nc.gpsimd.collective_compute(
                    "AllGather",
                    mybir.AluOpType.bypass,
                    replica_groups=[list(range(c.NCORES))],
                    ins=[tbl_loc[0][:].opt()],
                    outs=[tbl_sh[0][:].opt()],
                )

                def emit_layer1(b, pt):
                    o1t = wp.tile([HID, P], bf16, tag="o1t")
                    nc.scalar.activation(
                        out=o1t[:],
                        in_=pt[:],
                        func=mybir.ActivationFunctionType.Relu,
                        bias=b1c[:, 0:1],
                    )
                    ph2 = pd.tile([P, HID], f32, tag="pdense")
                    nc.tensor.matmul(
                        out=ph2[:], lhsT=o1t[:], rhs=w2[:], start=True, stop=True
                    )
                    nc.vector.tensor_scalar(
                        out=sck[1][:, b * HID : (b + 1) * HID],
                        in0=ph2[:],
                        scalar1=dinv_own[:, b : b + 1],
                        scalar2=None,
                        op0=mybir.AluOpType.mult,
                    )

                def emit_layer2(b, pt):
                    o2t = wp.tile([HID, P], bf16, tag="o1t")
                    nc.scalar.activation(
                        out=o2t[:],
                        in_=pt[:],
                        func=mybir.ActivationFunctionType.Relu,
                        bias=b2c[:, 0:1],
                    )
                    plg = pl.tile([P, 1], f32, tag="plogit")
                    nc.tensor.matmul(
                        out=plg[:], lhsT=o2t[:], rhs=wc[:], start=True, stop=True
                    )
                    nc.scalar.activation(
                        out=res_all[:, b : b + 1],
                        in_=plg[:],
                        func=mybir.ActivationFunctionType.Sigmoid,
                        bias=float(bc_val),
                    )

                with tc.tile_pool(name="msgs", bufs=3) as mp:
                    aggregate(
                        mp,
                        0,
                        emit_layer1,
                        emit_group=lambda g: store_group(1, g),
                    )

                    # ---- allgather table2, aggregate, classifier ----
                    nc.gpsimd.collective_compute(
                        "AllGather",
                        mybir.AluOpType.bypass,
                        replica_groups=[list(range(c.NCORES))],
                        ins=[tbl_loc[1][:].opt()],
                        outs=[tbl_sh[1][:].opt()],
                    )

                    aggregate(mp, 1, emit_layer2)

                nc.sync.dma_start(out=out_t[:, :], in_=res_all[:])

    nc.compile()
    return nc


def _to_bf16(a):
    import ml_dtypes

    return np.asarray(a, dtype=np.float32).astype(ml_dtypes.bfloat16)


def _make_in_maps(x, W1, W2, Wc, b1, b2, per_core, cfg):
    c = cfg
    iota = np.tile(np.arange(c.P, dtype=np.float32), (c.P, 1))
    in_maps = []
    for r in range(c.NCORES):
        xs = np.asarray(x, dtype=np.float32)[r * c.NSH : (r + 1) * c.NSH]
        xT = np.zeros((c.IN, c.NSH_PAD), np.float32)
        xT[:, : c.NSH] = xs.T
        pc = per_core[r]
        in_maps.append(
            {
                "xT": _to_bf16(xT),
                "idx16": pc["idx16"],
                "dstf": pc["dstf"],
                "dinvdst": pc["dinvdst"],
                "dinv_own": pc["dinv_own"],
                "iota": _to_bf16(iota),
                "diag_all": _to_bf16(pc["diag_all"]),
                "W1": _to_bf16(W1),
                "W2": _to_bf16(W2),
                "Wc": _to_bf16(Wc),
                "b1c": np.asarray(b1, np.float32).reshape(c.HID, 1),
                "b2c": np.asarray(b2, np.float32).reshape(c.HID, 1),
            }
        )
    return in_maps


def _assemble(results, cfg):
    c = cfg
    # out is [128, NB]: node b*128+p at [p, b]
    return np.concatenate(
        [results[r]["out"].T.reshape(-1, 1)[: c.NSH] for r in range(c.NCORES)],
        axis=0,
    )


def _make_runner(nc, in_maps, n_cores):
    """Build a reusable sharded PJRT callable over device-resident inputs.
    Returns (run_once() -> list[dict], time_steady(iters) -> ns_per_iter)."""
    import time as _t

    import jax
    import numpy as np_
    from jax.sharding import Mesh, NamedSharding, PartitionSpec
    from jax.experimental.shard_map import shard_map

    import concourse.mybir as mybir
    from concourse.bass2jax import _bass_exec_p, install_neuronx_cc_hook

    install_neuronx_cc_hook()

    partition_name = nc.partition_id_tensor.name if nc.partition_id_tensor else None
    in_names, out_names, out_avals, zero_outs = [], [], [], []
    for alloc in nc.m.functions[0].allocations:
        if not isinstance(alloc, mybir.MemoryLocationSet):
            continue
        name = alloc.memorylocations[0].name
        if alloc.kind == "ExternalInput":
            if name != partition_name:
                in_names.append(name)
        elif alloc.kind == "ExternalOutput":
            out_names.append(name)
            shape = tuple(alloc.tensor_shape)
            dtype = mybir.dt.np(alloc.dtype)
            out_avals.append(jax.core.ShapedArray(shape, dtype))
            zero_outs.append(np_.zeros(shape, dtype))
    n_params = len(in_names)
    all_in_names = in_names + out_names
    if partition_name is not None:
        all_in_names = all_in_names + [partition_name]

    def _body(*args):
        operands = list(args)
        if partition_name is not None:
            from concourse.bass2jax import partition_id_tensor

            operands.append(partition_id_tensor())
        outs = _bass_exec_p.bind(
            *operands,
            out_avals=tuple(out_avals),
            in_names=tuple(all_in_names),
            out_names=tuple(out_names),
            lowering_input_output_aliases=(),
            sim_require_finite=True,
            sim_require_nnan=True,
            nc=nc,
        )
        return tuple(outs)

    devices = jax.devices()[:n_cores]
    mesh = Mesh(np_.asarray(devices), ("core",))
    in_specs = (PartitionSpec("core"),) * (n_params + len(out_names))
    out_specs = (PartitionSpec("core"),) * len(out_names)
    sharded = jax.jit(
        shard_map(
            _body, mesh=mesh, in_specs=in_specs, out_specs=out_specs, check_rep=False
        ),
        keep_unused=True,
    )
    sh = NamedSharding(mesh, PartitionSpec("core"))
    concat_in = [
        jax.device_put(
            np_.concatenate([np_.asarray(in_maps[c][nm]) for c in range(n_cores)], 0),
            sh,
        )
        for nm in in_names
    ]
    concat_zeros = [
        jax.device_put(np_.zeros((n_cores * z.shape[0], *z.shape[1:]), z.dtype), sh)
        for z in zero_outs
    ]

    def run_once():
        out_arrs = sharded(*concat_in, *concat_zeros)
        jax.block_until_ready(out_arrs)
        return [
            {
                nm: np_.asarray(out_arrs[i]).reshape(n_cores, *out_avals[i].shape)[cc]
                for i, nm in enumerate(out_names)
            }
            for cc in range(n_cores)
        ]

    def time_steady(iters=10, warmup=3):
        for _ in range(warmup):
            jax.block_until_ready(sharded(*concat_in, *concat_zeros))
        t0 = _t.perf_counter()
        last = None
        for _ in range(iters):
            last = sharded(*concat_in, *concat_zeros)
        jax.block_until_ready(last)
        t1 = _t.perf_counter()
        return (t1 - t0) / iters * 1e9

    return run_once, time_steady


_CACHE = {}


def _get_built(edge_index, bc, repeat):
    """Cache (prep, compiled nc) keyed by edge data + bc + repeat."""
    edges = np.asarray(edge_index, dtype=np.int64)
    key = (hash(edges.tobytes()), float(bc), int(repeat))
    if key not in _CACHE:
        per_core, cnt_ch, base_ch, totc = _prep(edges, CFG_FULL)
        nc = _build(CFG_FULL, cnt_ch, base_ch, totc, float(bc), repeat=repeat)
        _CACHE[key] = (per_core, nc)
    return _CACHE[key]


def kernel(x, edge_index, W1, b1, W2, b2, Wc, bc):
    from concourse import bass_utils

    c = CFG_FULL
    bc_val = float(np.asarray(bc).reshape(-1)[0])
    per_core, nc = _get_built(edge_index, bc_val, 1)
    in_maps = _make_in_maps(x, W1, W2, Wc, b1, b2, per_core, c)
    res = bass_utils.run_bass_kernel_spmd(
        nc, in_maps, core_ids=list(range(c.NCORES)), trace=False
    )
    return _assemble(res.results, c)
